# revision 8
# baseline (speedup 1.0000x reference)
"""Trainium2 Bass kernel for nn_MultiHeadAttention_35837207118223.

Reference computation (faithfully reproduced, including softmax over the
HEAD axis):
    qh/kh/vh = split_heads(x @ W.T + b)           # [B,H,S,64]
    logits   = qh @ kh.T / 8 + mask * -1e9        # [B,H,S,S]
    attn     = softmax(logits, axis=1)            # over H (8 heads)
    ctx      = attn @ vh -> merge -> @ wo.T + bo
    returns (out, attn)

Key observations used:
  * mask is [B,1,1,S] (constant along the softmax axis H). In fp32,
    logit + (-1e9) rounds to exactly -1e9 (|logit| << ulp(1e9)=64), so at
    masked positions all 8 head-logits are identical => attn == 1/8 exactly.
    We therefore zero the masked COLUMNS of kh instead (masked logits = 0
    for every head => exp = 1 => attn = 1/8 exactly). No per-element mask
    work on the big [B,H,S,S] tensor.
  * 1/sqrt(64) is folded into wq/bq on the host (exact: power of two).

Sharding: data-parallel over batch, 2 batches per NeuronCore, 8 cores.
"""

import sys

sys.path.insert(0, "/opt/trn_rl_repo")

import numpy as np

import concourse.bacc as bacc
import concourse.mybir as mybir
from concourse.tile import TileContext
from concourse.bass_utils import run_bass_kernel_spmd

F32 = mybir.dt.float32
F16 = mybir.dt.float16
F32R = mybir.dt.float32r
AF = mybir.ActivationFunctionType
ALU = mybir.AluOpType

B, S, D, H = 16, 1024, 512, 8
NCORES = 8
BPC = B // NCORES  # batches per core
P = 128
DC = D // P  # 4 d-chunks
QT = S // P  # 8 q tiles
KT = S // P  # 8 k chunks

_STATE = {}


def _build_nc():
    nc = bacc.Bacc("TRN2", target_bir_lowering=False, debug=False, num_devices=NCORES)

    # ---- DRAM I/O (per-core shapes) ----
    qT = nc.dram_tensor("qT", [BPC, P, DC, S], F32R, kind="ExternalInput").ap()
    kT = nc.dram_tensor("kT", [BPC, P, DC, S], F32R, kind="ExternalInput").ap()
    vT = nc.dram_tensor("vT", [BPC, P, DC, S], F32R, kind="ExternalInput").ap()
    mskm = nc.dram_tensor("mskm", [BPC, P, S], F16, kind="ExternalInput").ap()
    wqT = nc.dram_tensor("wqT", [P, DC, D], F32R, kind="ExternalInput").ap()
    wkT = nc.dram_tensor("wkT", [P, DC, D], F32R, kind="ExternalInput").ap()
    wvT = nc.dram_tensor("wvT", [P, DC, D], F32R, kind="ExternalInput").ap()
    woT = nc.dram_tensor("woT", [P, DC, D], F16, kind="ExternalInput").ap()
    bq = nc.dram_tensor("bq", [P, DC], F32, kind="ExternalInput").ap()
    bk = nc.dram_tensor("bk", [P, DC], F32, kind="ExternalInput").ap()
    bv = nc.dram_tensor("bv", [P, D], F32, kind="ExternalInput").ap()
    bo = nc.dram_tensor("bo", [P, D], F32, kind="ExternalInput").ap()
    attn16 = nc.dram_tensor("attn16", [BPC, H, S, S], F16, kind="ExternalOutput").ap()
    outp = nc.dram_tensor("outp", [BPC, S, D], F32, kind="ExternalOutput").ap()

    with TileContext(nc) as tc:
        with (
            tc.tile_pool(name="wp", bufs=1) as wp,
            tc.tile_pool(name="inp", bufs=2) as inp,
            tc.tile_pool(name="proj", bufs=1) as proj,
            tc.tile_pool(name="ep", bufs=2) as ep,
            tc.tile_pool(name="sump", bufs=2) as sump,
            tc.tile_pool(name="sump2", bufs=2) as sump2,
            tc.tile_pool(name="sp", bufs=1) as sp,
            tc.tile_pool(name="lp", bufs=1) as lp,
            tc.tile_pool(name="rp", bufs=2) as rp,
            tc.tile_pool(name="atp", bufs=2) as atp,
            tc.tile_pool(name="tp", bufs=2) as tp,
            tc.tile_pool(name="ob", bufs=2) as ob,
            tc.tile_pool(name="ps_sm", bufs=2, space="PSUM") as ps_sm,
            tc.tile_pool(name="ps_lg", bufs=2, space="PSUM") as ps_lg,
            tc.tile_pool(name="ps_cx", bufs=2, space="PSUM") as ps_cx,
        ):
            # ---- persistent weights ----
            wqT_t = wp.tile([P, DC, D], F32R)
            nc.sync.dma_start(wqT_t[:], wqT[:])
            wkT_t = wp.tile([P, DC, D], F32R)
            nc.sync.dma_start(wkT_t[:], wkT[:])
            wvT_t = wp.tile([P, DC, D], F32R)
            nc.sync.dma_start(wvT_t[:], wvT[:])
            woT_t = wp.tile([P, DC, D], F16)
            nc.sync.dma_start(woT_t[:], woT[:])
            bq_t = wp.tile([P, DC], F32)
            nc.sync.dma_start(bq_t[:], bq[:])
            bk_t = wp.tile([P, DC], F32)
            nc.sync.dma_start(bk_t[:], bk[:])
            bv_t = wp.tile([P, D], F32)
            nc.sync.dma_start(bv_t[:], bv[:])
            bo_t = wp.tile([P, D], F32)
            nc.sync.dma_start(bo_t[:], bo[:])

            for b in range(BPC):
                # ---- load activations (d-major) ----
                msk = inp.tile([P, S], F16, tag="msk")
                nc.sync.dma_start(msk[:], mskm[b])

                qhT = proj.tile([P, DC, S], F32R, tag="qhT")
                khT = proj.tile([P, DC, S], F32R, tag="khT")
                vh = proj.tile([P, KT, D], F16, tag="vh")
                ctxT = proj.tile([P, DC, S], F16, tag="ctxT")

                # ---- projections (q, then k, then v, so "in" slots recycle) ----
                qin = inp.tile([P, DC, S], F32R, tag="in")
                nc.sync.dma_start(qin[:], qT[b])
                for c in range(DC):
                    for nh in range(2):
                        sl = slice(nh * 512, nh * 512 + 512)
                        ps = ps_sm.tile([P, 512], F32, tag="sm")
                        for kc in range(DC):
                            nc.tensor.matmul(
                                ps[:],
                                lhsT=wqT_t[:, kc, c * P : (c + 1) * P],
                                rhs=qin[:, kc, sl],
                                start=(kc == 0),
                                stop=(kc == DC - 1),
                            )
                        nc.vector.tensor_scalar(
                            qhT[:, c, sl], ps[:], bq_t[:, c : c + 1], None, ALU.add
                        )

                kin = inp.tile([P, DC, S], F32R, tag="in")
                nc.sync.dma_start(kin[:], kT[b])
                for c in range(DC):
                    for nh in range(2):
                        sl = slice(nh * 512, nh * 512 + 512)
                        ps2 = ps_sm.tile([P, 512], F32, tag="sm")
                        for kc in range(DC):
                            nc.tensor.matmul(
                                ps2[:],
                                lhsT=wkT_t[:, kc, c * P : (c + 1) * P],
                                rhs=kin[:, kc, sl],
                                start=(kc == 0),
                                stop=(kc == DC - 1),
                            )
                        # fused: khT = (psum + bk) * mask01  (masked cols -> 0)
                        nc.vector.scalar_tensor_tensor(
                            khT[:, c, sl],
                            ps2[:],
                            bk_t[:, c : c + 1],
                            msk[:, sl],
                            ALU.add,
                            ALU.mult,
                        )

                # vh in s-major [s_p, s_chunk, dout], fp16
                vin = inp.tile([P, DC, S], F32R, tag="in")
                nc.sync.dma_start(vin[:], vT[b])
                for st in range(KT):
                    ps = ps_sm.tile([P, 512], F32, tag="sm")
                    for kc in range(DC):
                        nc.tensor.matmul(
                            ps[:],
                            lhsT=vin[:, kc, st * P : (st + 1) * P],
                            rhs=wvT_t[:, kc, :],
                            start=(kc == 0),
                            stop=(kc == DC - 1),
                        )
                    nc.vector.tensor_add(vh[:, st, :], ps[:], bv_t[:])

                # ---- attention ----
                for qu in range(4):  # q quarters (2 q-tiles each)
                    aT = [
                        tp.tile([P, 4, KT, 256], F16, tag="aT", name=f"aT0_{b}_{qu}"),
                        tp.tile([P, 4, KT, 256], F16, tag="aT", name=f"aT1_{b}_{qu}"),
                    ]
                    for qq in range(2):
                        qt = 2 * qu + qq
                        qsl = slice(qt * P, (qt + 1) * P)
                        e = ep.tile([P, H, S], F16, tag="e")
                        for h in range(H):
                            hc, hr = h // 2, (h % 2) * 64
                            pl = ps_lg.tile([P, S], F32, tag="lg")
                            for kh in range(2):
                                ksl = slice(kh * 512, kh * 512 + 512)
                                nc.tensor.matmul(
                                    pl[:, ksl],
                                    lhsT=qhT[hr : hr + 64, hc, qsl],
                                    rhs=khT[hr : hr + 64, hc, ksl],
                                    start=True,
                                    stop=True,
                                )
                            nc.scalar.activation(e[:, h, :], pl[:], AF.Exp)
                        # sum over heads (tree, fp16 2x mode)
                        p0 = sump.tile([P, S], F16, tag="pair")
                        nc.vector.tensor_add(p0[:], e[:, 0, :], e[:, 1, :])
                        p1 = sump.tile([P, S], F16, tag="pair")
                        nc.vector.tensor_add(p1[:], e[:, 2, :], e[:, 3, :])
                        q0 = sump2.tile([P, S], F16, tag="pair2")
                        nc.vector.tensor_add(q0[:], p0[:], p1[:])
                        p2 = sump.tile([P, S], F16, tag="pair")
                        nc.vector.tensor_add(p2[:], e[:, 4, :], e[:, 5, :])
                        p3 = sump.tile([P, S], F16, tag="pair")
                        nc.vector.tensor_add(p3[:], e[:, 6, :], e[:, 7, :])
                        q1 = sump2.tile([P, S], F16, tag="pair2")
                        nc.vector.tensor_add(q1[:], p2[:], p3[:])
                        s_t = sp.tile([P, S], F16, tag="s")
                        nc.vector.tensor_add(s_t[:], q0[:], q1[:])
                        # r = 1/s via exp(-ln(s)) on ACT (exp/ln share a table set)
                        lnS = lp.tile([P, S], F32, tag="ln")
                        nc.scalar.activation(lnS[:], s_t[:], AF.Ln)
                        r_t = rp.tile([P, S], F16, tag="r")
                        nc.scalar.activation(r_t[:], lnS[:], AF.Exp, scale=-1.0)
                        # attn = e * r; write to HBM (fp16) + transpose for ctx
                        for h in range(H):
                            at = atp.tile([P, S], F16, tag="at")
                            nc.vector.tensor_mul(at[:], e[:, h, :], r_t[:])
                            nc.sync.dma_start(attn16[b, h, qsl, :], at[:])
                            dst = aT[h // 4]
                            for kc in range(KT):
                                nc.sync.dma_start_transpose(
                                    dst[:, h % 4, kc, qq * P : (qq + 1) * P],
                                    at[:, kc * P : (kc + 1) * P],
                                )
                    # ---- ctx for this quarter: ctxT[d, q] per head pair ----
                    for hp in range(4):
                        pc = ps_cx.tile([P, 256], F32, tag="cx")
                        for r2 in range(2):
                            h = 2 * hp + r2
                            dst = aT[h // 4]
                            for kc in range(KT):
                                nc.tensor.matmul(
                                    pc[r2 * 64 : r2 * 64 + 64, :],
                                    lhsT=vh[:, kc, h * 64 : (h + 1) * 64],
                                    rhs=dst[:, h % 4, kc, :],
                                    start=(kc == 0),
                                    stop=(kc == KT - 1),
                                )
                        nc.vector.tensor_copy(
                            ctxT[:, hp, qu * 256 : (qu + 1) * 256], pc[:]
                        )
                    # ---- output projection for the quarter's q-tiles ----
                    for qq in range(2):
                        qt = 2 * qu + qq
                        qsl = slice(qt * P, (qt + 1) * P)
                        po = ps_sm.tile([P, 512], F32, tag="sm")
                        for dc in range(DC):
                            nc.tensor.matmul(
                                po[:],
                                lhsT=ctxT[:, dc, qsl],
                                rhs=woT_t[:, dc, :],
                                start=(dc == 0),
                                stop=(dc == DC - 1),
                            )
                        osb = ob.tile([P, 512], F32, tag="osb")
                        nc.vector.tensor_add(osb[:], po[:], bo_t[:])
                        nc.sync.dma_start(outp[b, qsl, :], osb[:])

    nc.compile()
    return nc


def _get_nc():
    if "nc" not in _STATE:
        _STATE["nc"] = _build_nc()
    return _STATE["nc"]


def _prep_dmajor(x):
    # [B, S, D] -> [B, P, DC, S] with d = c*128 + p
    xt = np.ascontiguousarray(x.transpose(0, 2, 1))  # [B, D, S]
    return np.ascontiguousarray(
        xt.reshape(B, DC, P, S).transpose(0, 2, 1, 3)
    )  # [B, P, DC, S]


def _prep_w(w):
    # torch Linear weight [dout, din] -> lhsT layout [P, DC, dout], din = c*128+p
    wt = np.ascontiguousarray(w.T)  # [din, dout]
    return np.ascontiguousarray(wt.reshape(DC, P, D).transpose(1, 0, 2))


def kernel(q, k, v, mask, wq_w, wq_b, wk_w, wk_b, wv_w, wv_b, wo_w, wo_b):
    nc = _get_nc()

    scale = np.float32(1.0 / 8.0)  # 1/sqrt(64), exact power of two
    qT = _prep_dmajor(np.asarray(q, np.float32))
    kT = _prep_dmajor(np.asarray(k, np.float32))
    vT = _prep_dmajor(np.asarray(v, np.float32))
    wqTp = _prep_w(np.asarray(wq_w, np.float32) * scale)
    wkTp = _prep_w(np.asarray(wk_w, np.float32))
    wvTp = _prep_w(np.asarray(wv_w, np.float32))
    woTp = _prep_w(np.asarray(wo_w, np.float32)).astype(np.float16)
    bqp = np.ascontiguousarray((np.asarray(wq_b, np.float32) * scale).reshape(DC, P).T)
    bkp = np.ascontiguousarray(np.asarray(wk_b, np.float32).reshape(DC, P).T)
    bvp = np.ascontiguousarray(
        np.broadcast_to(np.asarray(wv_b, np.float32), (P, D))
    )
    bop = np.ascontiguousarray(
        np.broadcast_to(np.asarray(wo_b, np.float32), (P, D))
    )
    # mask multiplier: 1.0 where unmasked, 0.0 where masked (mask==1)
    m01 = (np.asarray(mask)[:, 0, 0, :] == 0).astype(np.float16)  # [B, S]
    mskm = np.ascontiguousarray(np.broadcast_to(m01[:, None, :], (B, P, S)))

    in_maps = []
    for c in range(NCORES):
        b0 = c * BPC
        in_maps.append(
            {
                "qT": qT[b0 : b0 + BPC],
                "kT": kT[b0 : b0 + BPC],
                "vT": vT[b0 : b0 + BPC],
                "mskm": mskm[b0 : b0 + BPC],
                "wqT": wqTp,
                "wkT": wkTp,
                "wvT": wvTp,
                "woT": woTp,
                "bq": bqp,
                "bk": bkp,
                "bv": bvp,
                "bo": bop,
            }
        )

    res = run_bass_kernel_spmd(
        nc, in_maps, list(range(NCORES)), trace=_STATE.get("trace", False)
    )
    _STATE["last_res"] = res
    out = np.concatenate([res.results[i]["outp"] for i in range(NCORES)], axis=0)
    attn = np.concatenate(
        [res.results[i]["attn16"] for i in range(NCORES)], axis=0
    ).astype(np.float32)
    return out, attn


# revision 16
# speedup vs baseline: 2.1901x; 2.1901x over previous
"""Trainium2 Bass kernel for nn_MultiHeadAttention_35837207118223.

Reference computation (faithfully reproduced, including softmax over the
HEAD axis):
    qh/kh/vh = split_heads(x @ W.T + b)           # [B,H,S,64]
    logits   = qh @ kh.T / 8 + mask * -1e9        # [B,H,S,S]
    attn     = softmax(logits, axis=1)            # over H (8 heads)
    ctx      = attn @ vh -> merge -> @ wo.T + bo
    returns (out, attn)

Key observations used:
  * mask is [B,1,1,S] (constant along the softmax axis H). In fp32,
    logit + (-1e9) rounds to exactly -1e9 (|logit| << ulp(1e9)=64), so at
    masked positions all 8 head-logits are identical => attn == 1/8 exactly.
    We therefore zero the masked COLUMNS of kh instead (masked logits = 0
    for every head => exp = 1 => attn = 1/8 exactly). No per-element mask
    work on the big [B,H,S,S] tensor.
  * 1/sqrt(64) is folded into wq/bq on the host (exact: power of two).

Sharding: data-parallel over batch, 2 batches per NeuronCore, 8 cores.
"""

import sys

sys.path.insert(0, "/opt/trn_rl_repo")

import numpy as np

import concourse.bacc as bacc
import concourse.mybir as mybir
from concourse.tile import TileContext
from concourse.bass_utils import run_bass_kernel_spmd

F32 = mybir.dt.float32
F16 = mybir.dt.float16
F32R = mybir.dt.float32r
AF = mybir.ActivationFunctionType
ALU = mybir.AluOpType

B, S, D, H = 16, 1024, 512, 8
NCORES = 8
BPC = B // NCORES  # batches per core
P = 128
DC = D // P  # 4 d-chunks
QT = S // P  # 8 q tiles
KT = S // P  # 8 k chunks

_STATE = {}


def _build_nc():
    nc = bacc.Bacc("TRN2", target_bir_lowering=False, debug=False, num_devices=NCORES)

    # ---- DRAM I/O (per-core shapes) ----
    qT = nc.dram_tensor("qT", [BPC, P, DC, S], F32R, kind="ExternalInput").ap()
    kT = nc.dram_tensor("kT", [BPC, P, DC, S], F32R, kind="ExternalInput").ap()
    vT = nc.dram_tensor("vT", [BPC, P, DC, S], F32R, kind="ExternalInput").ap()
    mskm = nc.dram_tensor("mskm", [BPC, P, S], F16, kind="ExternalInput").ap()
    wqT = nc.dram_tensor("wqT", [P, DC, D], F32R, kind="ExternalInput").ap()
    wkT = nc.dram_tensor("wkT", [P, DC, D], F32R, kind="ExternalInput").ap()
    wvT = nc.dram_tensor("wvT", [P, DC, D], F32R, kind="ExternalInput").ap()
    woT = nc.dram_tensor("woT", [P, DC, D], F16, kind="ExternalInput").ap()
    bq = nc.dram_tensor("bq", [P, DC], F32, kind="ExternalInput").ap()
    bk = nc.dram_tensor("bk", [P, DC], F32, kind="ExternalInput").ap()
    bv = nc.dram_tensor("bv", [P, D], F32, kind="ExternalInput").ap()
    bo = nc.dram_tensor("bo", [P, D], F32, kind="ExternalInput").ap()
    attn16 = nc.dram_tensor("attn16", [BPC, H, S, S], F16, kind="ExternalOutput").ap()
    outp = nc.dram_tensor("outp", [BPC, S, D], F32, kind="ExternalOutput").ap()

    with TileContext(nc) as tc:
        with (
            tc.tile_pool(name="wp", bufs=1) as wp,
            tc.tile_pool(name="inp", bufs=2) as inp,
            tc.tile_pool(name="proj", bufs=1) as proj,
            tc.tile_pool(name="ep", bufs=2) as ep,
            tc.tile_pool(name="sump", bufs=2) as sump,
            tc.tile_pool(name="sump2", bufs=2) as sump2,
            tc.tile_pool(name="sp", bufs=1) as sp,
            tc.tile_pool(name="lp", bufs=1) as lp,
            tc.tile_pool(name="rp", bufs=2) as rp,
            tc.tile_pool(name="atp", bufs=2) as atp,
            tc.tile_pool(name="tp", bufs=2) as tp,
            tc.tile_pool(name="ob", bufs=2) as ob,
            tc.tile_pool(name="ps_sm", bufs=2, space="PSUM") as ps_sm,
            tc.tile_pool(name="ps_lg", bufs=2, space="PSUM") as ps_lg,
            tc.tile_pool(name="ps_cx", bufs=2, space="PSUM") as ps_cx,
        ):
            # ---- persistent weights ----
            wqT_t = wp.tile([P, DC, D], F32R)
            nc.sync.dma_start(wqT_t[:], wqT[:])
            wkT_t = wp.tile([P, DC, D], F32R)
            nc.sync.dma_start(wkT_t[:], wkT[:])
            wvT_t = wp.tile([P, DC, D], F32R)
            nc.sync.dma_start(wvT_t[:], wvT[:])
            woT_t = wp.tile([P, DC, D], F16)
            nc.sync.dma_start(woT_t[:], woT[:])
            bq_t = wp.tile([P, DC], F32)
            nc.sync.dma_start(bq_t[:], bq[:])
            bk_t = wp.tile([P, DC], F32)
            nc.sync.dma_start(bk_t[:], bk[:])
            bv_t = wp.tile([P, D], F32)
            nc.sync.dma_start(bv_t[:], bv[:])
            bo_t = wp.tile([P, D], F32)
            nc.sync.dma_start(bo_t[:], bo[:])

            for b in range(BPC):
                # ---- load activations (d-major) ----
                msk = inp.tile([P, S], F16, tag="msk")
                nc.sync.dma_start(msk[:], mskm[b])

                qhT = proj.tile([P, DC, S], F32R, tag="qhT")
                khT = proj.tile([P, DC, S], F32R, tag="khT")
                vh = proj.tile([P, KT, D], F16, tag="vh")
                ctxT = proj.tile([P, DC, S], F16, tag="ctxT")

                # ---- projections (q, then k, then v, so "in" slots recycle) ----
                qin = inp.tile([P, DC, S], F32R, tag="in")
                nc.sync.dma_start(qin[:], qT[b])
                for c in range(DC):
                    for nh in range(2):
                        sl = slice(nh * 512, nh * 512 + 512)
                        ps = ps_sm.tile([P, 512], F32, tag="sm")
                        for kc in range(DC):
                            nc.tensor.matmul(
                                ps[:],
                                lhsT=wqT_t[:, kc, c * P : (c + 1) * P],
                                rhs=qin[:, kc, sl],
                                start=(kc == 0),
                                stop=(kc == DC - 1),
                            )
                        nc.vector.tensor_scalar(
                            qhT[:, c, sl], ps[:], bq_t[:, c : c + 1], None, ALU.add
                        )

                kin = inp.tile([P, DC, S], F32R, tag="in")
                nc.sync.dma_start(kin[:], kT[b])
                for c in range(DC):
                    for nh in range(2):
                        sl = slice(nh * 512, nh * 512 + 512)
                        ps2 = ps_sm.tile([P, 512], F32, tag="sm")
                        for kc in range(DC):
                            nc.tensor.matmul(
                                ps2[:],
                                lhsT=wkT_t[:, kc, c * P : (c + 1) * P],
                                rhs=kin[:, kc, sl],
                                start=(kc == 0),
                                stop=(kc == DC - 1),
                            )
                        # fused: khT = (psum + bk) * mask01  (masked cols -> 0)
                        nc.vector.scalar_tensor_tensor(
                            khT[:, c, sl],
                            ps2[:],
                            bk_t[:, c : c + 1],
                            msk[:, sl],
                            ALU.add,
                            ALU.mult,
                        )

                # vh in s-major [s_p, s_chunk, dout], fp16
                vin = inp.tile([P, DC, S], F32R, tag="in")
                nc.sync.dma_start(vin[:], vT[b])
                for st in range(KT):
                    ps = ps_sm.tile([P, 512], F32, tag="sm")
                    for kc in range(DC):
                        nc.tensor.matmul(
                            ps[:],
                            lhsT=vin[:, kc, st * P : (st + 1) * P],
                            rhs=wvT_t[:, kc, :],
                            start=(kc == 0),
                            stop=(kc == DC - 1),
                        )
                    nc.vector.tensor_add(vh[:, st, :], ps[:], bv_t[:])

                # ---- attention ----
                for qu in range(4):  # q quarters (2 q-tiles each)
                    # [k_part, head%4, k_chunk, q_block, q] — ctx rhs
                    # [:, hh, kc, :, :] is contiguous [P, 256]; the batched
                    # transpose writes the gapped 3D view [:, hh, :, qq, :]
                    aT = [
                        tp.tile([P, 4, KT, 2, P], F16, tag="aT", name=f"aT0_{b}_{qu}"),
                        tp.tile([P, 4, KT, 2, P], F16, tag="aT", name=f"aT1_{b}_{qu}"),
                    ]
                    for qq in range(2):
                        qt = 2 * qu + qq
                        qsl = slice(qt * P, (qt + 1) * P)
                        e = ep.tile([P, H, S], F16, tag="e")
                        for h in range(H):
                            hc, hr = h // 2, (h % 2) * 64
                            pl = ps_lg.tile([P, S], F32, tag="lg")
                            for kh in range(2):
                                ksl = slice(kh * 512, kh * 512 + 512)
                                nc.tensor.matmul(
                                    pl[:, ksl],
                                    lhsT=qhT[hr : hr + 64, hc, qsl],
                                    rhs=khT[hr : hr + 64, hc, ksl],
                                    start=True,
                                    stop=True,
                                )
                            nc.scalar.activation(e[:, h, :], pl[:], AF.Exp)
                        # sum over heads (tree, fp16 2x mode)
                        p0 = sump.tile([P, S], F16, tag="pair")
                        nc.vector.tensor_add(p0[:], e[:, 0, :], e[:, 1, :])
                        p1 = sump.tile([P, S], F16, tag="pair")
                        nc.vector.tensor_add(p1[:], e[:, 2, :], e[:, 3, :])
                        q0 = sump2.tile([P, S], F16, tag="pair2")
                        nc.vector.tensor_add(q0[:], p0[:], p1[:])
                        p2 = sump.tile([P, S], F16, tag="pair")
                        nc.vector.tensor_add(p2[:], e[:, 4, :], e[:, 5, :])
                        p3 = sump.tile([P, S], F16, tag="pair")
                        nc.vector.tensor_add(p3[:], e[:, 6, :], e[:, 7, :])
                        q1 = sump2.tile([P, S], F16, tag="pair2")
                        nc.vector.tensor_add(q1[:], p2[:], p3[:])
                        s_t = sp.tile([P, S], F16, tag="s")
                        nc.vector.tensor_add(s_t[:], q0[:], q1[:])
                        # r = 1/s via exp(-ln(s)) on ACT (exp/ln share a table set)
                        lnS = lp.tile([P, S], F32, tag="ln")
                        nc.scalar.activation(lnS[:], s_t[:], AF.Ln)
                        r_t = rp.tile([P, S], F16, tag="r")
                        nc.scalar.activation(r_t[:], lnS[:], AF.Exp, scale=-1.0)
                        # attn = e * r; write to HBM (fp16) + transpose for ctx
                        for h in range(H):
                            at = atp.tile([P, S], F16, tag="at")
                            nc.vector.tensor_mul(at[:], e[:, h, :], r_t[:])
                            nc.sync.dma_start(attn16[b, h, qsl, :], at[:])
                            # one batched xbar transpose: [128,1024] -> [128,8,128]
                            nc.sync.dma_start_transpose(
                                aT[h // 4][:, h % 4, :, qq, :], at[:]
                            )
                    # ---- ctx for this quarter: ctxT[d, q] per head pair ----
                    for hp in range(4):
                        pc = ps_cx.tile([P, 256], F32, tag="cx")
                        for r2 in range(2):
                            h = 2 * hp + r2
                            dst = aT[h // 4]
                            for kc in range(KT):
                                nc.tensor.matmul(
                                    pc[r2 * 64 : r2 * 64 + 64, :],
                                    lhsT=vh[:, kc, h * 64 : (h + 1) * 64],
                                    rhs=dst[:, h % 4, kc, :, :],
                                    start=(kc == 0),
                                    stop=(kc == KT - 1),
                                )
                        nc.vector.tensor_copy(
                            ctxT[:, hp, qu * 256 : (qu + 1) * 256], pc[:]
                        )
                    # ---- output projection for the quarter's q-tiles ----
                    for qq in range(2):
                        qt = 2 * qu + qq
                        qsl = slice(qt * P, (qt + 1) * P)
                        po = ps_sm.tile([P, 512], F32, tag="sm")
                        for dc in range(DC):
                            nc.tensor.matmul(
                                po[:],
                                lhsT=ctxT[:, dc, qsl],
                                rhs=woT_t[:, dc, :],
                                start=(dc == 0),
                                stop=(dc == DC - 1),
                            )
                        osb = ob.tile([P, 512], F32, tag="osb")
                        nc.vector.tensor_add(osb[:], po[:], bo_t[:])
                        nc.sync.dma_start(outp[b, qsl, :], osb[:])

    nc.compile()
    return nc


def _get_nc():
    if "nc" not in _STATE:
        _STATE["nc"] = _build_nc()
    return _STATE["nc"]


def _prep_dmajor(x):
    # [B, S, D] -> [B, P, DC, S] with d = c*128 + p
    xt = np.ascontiguousarray(x.transpose(0, 2, 1))  # [B, D, S]
    return np.ascontiguousarray(
        xt.reshape(B, DC, P, S).transpose(0, 2, 1, 3)
    )  # [B, P, DC, S]


def _prep_w(w):
    # torch Linear weight [dout, din] -> lhsT layout [P, DC, dout], din = c*128+p
    wt = np.ascontiguousarray(w.T)  # [din, dout]
    return np.ascontiguousarray(wt.reshape(DC, P, D).transpose(1, 0, 2))


def kernel(q, k, v, mask, wq_w, wq_b, wk_w, wk_b, wv_w, wv_b, wo_w, wo_b):
    nc = _get_nc()

    scale = np.float32(1.0 / 8.0)  # 1/sqrt(64), exact power of two
    qT = _prep_dmajor(np.asarray(q, np.float32))
    kT = _prep_dmajor(np.asarray(k, np.float32))
    vT = _prep_dmajor(np.asarray(v, np.float32))
    wqTp = _prep_w(np.asarray(wq_w, np.float32) * scale)
    wkTp = _prep_w(np.asarray(wk_w, np.float32))
    wvTp = _prep_w(np.asarray(wv_w, np.float32))
    woTp = _prep_w(np.asarray(wo_w, np.float32)).astype(np.float16)
    bqp = np.ascontiguousarray((np.asarray(wq_b, np.float32) * scale).reshape(DC, P).T)
    bkp = np.ascontiguousarray(np.asarray(wk_b, np.float32).reshape(DC, P).T)
    bvp = np.ascontiguousarray(
        np.broadcast_to(np.asarray(wv_b, np.float32), (P, D))
    )
    bop = np.ascontiguousarray(
        np.broadcast_to(np.asarray(wo_b, np.float32), (P, D))
    )
    # mask multiplier: 1.0 where unmasked, 0.0 where masked (mask==1)
    m01 = (np.asarray(mask)[:, 0, 0, :] == 0).astype(np.float16)  # [B, S]
    mskm = np.ascontiguousarray(np.broadcast_to(m01[:, None, :], (B, P, S)))

    in_maps = []
    for c in range(NCORES):
        b0 = c * BPC
        in_maps.append(
            {
                "qT": qT[b0 : b0 + BPC],
                "kT": kT[b0 : b0 + BPC],
                "vT": vT[b0 : b0 + BPC],
                "mskm": mskm[b0 : b0 + BPC],
                "wqT": wqTp,
                "wkT": wkTp,
                "wvT": wvTp,
                "woT": woTp,
                "bq": bqp,
                "bk": bkp,
                "bv": bvp,
                "bo": bop,
            }
        )

    res = run_bass_kernel_spmd(
        nc, in_maps, list(range(NCORES)), trace=_STATE.get("trace", False)
    )
    _STATE["last_res"] = res
    out = np.concatenate([res.results[i]["outp"] for i in range(NCORES)], axis=0)
    attn = np.concatenate(
        [res.results[i]["attn16"] for i in range(NCORES)], axis=0
    ).astype(np.float32)
    return out, attn
